# revision 1
# baseline (speedup 1.0000x reference)
"""TransformerConv 2-layer GNN encoder on 8 Trainium2 NeuronCores (Bass/Tile).

Strategy (graph-partition parallel, full tables local to each core):
  - Nodes padded 50000 -> 50176 = 8 cores x 49 tiles x 128.
  - Each core owns the 49 consecutive node tiles of its rank as TARGETS; edges
    are assigned to the core owning their dst, sorted by dst, packed into
    128-edge chunks per tile (chunk counts equalized across cores so the SPMD
    program is identical).
  - Phase A (per layer): every core computes the full q|k|v table [50176,384]
    with PE matmuls (transpose trick), so per-edge gathers stay core-local.
  - Phase B (per layer): per chunk: indirect-DMA gathers of k[src],v[src],
    q[dst]; edge features ea@We on PE; per-edge attention logits on DVE;
    exp on ACT; segment softmax-sum via one-hot matmul into PSUM accumulated
    over the tile's chunks; fused divide on evac; skip connection via PE.
  - One AllGather of h between the layers.
Softmax note: segment-max subtraction is skipped (alphas are O(1); exact
softmax invariance), and the division is applied after summation - both
match the reference to fp32 rounding.
"""
import numpy as np

P = 128
N = 50000
NP_ = 50176
TILES = 392
NCORES = 8
TPC = TILES // NCORES          # 49 tiles per core
NLC = TPC * P                  # 6272 local nodes
NODE_DIM = 128
EDGE_DIM = 16
HID = 128
DSTREL_PAD = 200.0
SLAB = 64                      # index columns loaded per slab DMA


# ----------------------------------------------------------------- host prep
def _prep(ei, ea):
    src = np.asarray(ei[0], dtype=np.int64)
    dst = np.asarray(ei[1], dtype=np.int64)
    ea = np.asarray(ea, dtype=np.float32)

    order = np.argsort(dst, kind="stable")
    src_s, dst_s, ea_s = src[order], dst[order], ea[order]

    tile_of = dst_s // P
    cnt = np.bincount(tile_of, minlength=TILES)
    C = (cnt + P - 1) // P
    Cloc = np.maximum(C.reshape(NCORES, TPC).max(axis=0), 1)   # [TPC] chunks per tile
    NCH = int(Cloc.sum())
    off = np.zeros(TPC, dtype=np.int64)
    off[1:] = np.cumsum(Cloc)[:-1]

    tile_starts = np.searchsorted(tile_of, np.arange(TILES))
    tile_ends = np.searchsorted(tile_of, np.arange(TILES), side="right")
    cores = []
    for c in range(NCORES):
        nslot = NCH * P
        src_sl = np.zeros(nslot, dtype=np.int32)
        qdst_sl = np.zeros(nslot, dtype=np.int32)
        drel_sl = np.full(nslot, DSTREL_PAD, dtype=np.float32)
        ea_sl = np.zeros((nslot, EDGE_DIM), dtype=np.float32)
        for tl in range(TPC):
            tg = c * TPC + tl
            a, b = tile_starts[tg], tile_ends[tg]
            if b == a:
                continue
            s0 = off[tl] * P
            src_sl[s0:s0 + b - a] = src_s[a:b]
            qdst_sl[s0:s0 + b - a] = dst_s[a:b]
            drel_sl[s0:s0 + b - a] = (dst_s[a:b] - tg * P).astype(np.float32)
            ea_sl[s0:s0 + b - a] = ea_s[a:b]
        cores.append(dict(
            srcT=np.ascontiguousarray(src_sl.reshape(NCH, P).T),
            qdstT=np.ascontiguousarray(qdst_sl.reshape(NCH, P).T),
            dstrelT=np.ascontiguousarray(drel_sl.reshape(NCH, P).T),
            eaT=np.ascontiguousarray(ea_sl.T),
        ))
    return cores, Cloc, off, NCH


# ------------------------------------------------------- walrus wait legalize
def _legalize_waits(nc):
    import concourse.mybir as mybir
    k = 0
    for bb in nc.main_func.blocks:
        il = bb.instructions
        new = []
        for ins in il:
            si = ins.sync_info
            if si is not None and len(si.on_wait) > 1:
                waits = list(si.on_wait)
                for w in waits[:-1]:
                    nop = mybir.InstNoOp(name=f"wn{k}-{ins.name}", ins=[], outs=[])
                    k += 1
                    nop.engine = ins.engine
                    nop.sync_info = mybir.SyncInfo(on_wait=[w], on_update=[])
                    new.append(nop)
                ins.sync_info = mybir.SyncInfo(on_wait=[waits[-1]],
                                               on_update=list(si.on_update))
            new.append(ins)
        il[:] = new


# ------------------------------------------------------------- device program
def _build(Cloc, off, NCH):
    import concourse.bass as bass
    import concourse.mybir as mybir
    import concourse.tile as tile
    from concourse.masks import make_identity
    f32 = mybir.dt.float32
    i32 = mybir.dt.int32
    Alu = mybir.AluOpType
    Act = mybir.ActivationFunctionType

    nc = bass.Bass()
    x_full = nc.declare_dram_parameter("x_full", [NP_, NODE_DIM], f32, isOutput=False)
    x_local = nc.declare_dram_parameter("x_local", [NLC, NODE_DIM], f32, isOutput=False)
    srcT = nc.declare_dram_parameter("srcT", [P, NCH], i32, isOutput=False)
    qdstT = nc.declare_dram_parameter("qdstT", [P, NCH], i32, isOutput=False)
    dstrelT = nc.declare_dram_parameter("dstrelT", [P, NCH], f32, isOutput=False)
    eaT = nc.declare_dram_parameter("eaT", [EDGE_DIM, NCH * P], f32, isOutput=False)
    Wqkv1 = nc.declare_dram_parameter("Wqkv1", [NODE_DIM, 3 * HID], f32, isOutput=False)
    We1 = nc.declare_dram_parameter("We1", [EDGE_DIM, HID], f32, isOutput=False)
    Ws1 = nc.declare_dram_parameter("Ws1", [NODE_DIM, HID], f32, isOutput=False)
    Wqkv2 = nc.declare_dram_parameter("Wqkv2", [HID, 3 * HID], f32, isOutput=False)
    We2 = nc.declare_dram_parameter("We2", [EDGE_DIM, HID], f32, isOutput=False)
    Ws2 = nc.declare_dram_parameter("Ws2", [HID, HID], f32, isOutput=False)
    out = nc.declare_dram_parameter("out", [NLC, HID], f32, isOutput=True)

    qkv_tab = nc.dram_tensor("qkv_tab", [NP_, 3 * HID], f32, kind="Internal")
    h_local = nc.dram_tensor("h_local", [NLC, HID], f32, kind="Internal")
    h_full = nc.dram_tensor("h_full", [NP_, HID], f32, kind="Internal")

    # ---------------- phase A: qkv table from node features
    def qkv_phase(xin, wqkv):
        with tile.TileContext(nc) as tc:
            with tc.tile_pool(name="qk_c", bufs=1) as cst, \
                 tc.tile_pool(name="qk_s", bufs=3) as pool, \
                 tc.tile_pool(name="qk_p", bufs=2, space="PSUM") as psp, \
                 tc.tile_pool(name="qk_p2", bufs=2, space="PSUM") as psp2:
                idt = cst.tile([P, P], f32)
                make_identity(nc, idt[:])
                wt = cst.tile([NODE_DIM, 3 * HID], f32)
                nc.sync.dma_start(out=wt[:], in_=wqkv[:])
                for i in range(NP_ // P):
                    xt = pool.tile([P, NODE_DIM], f32, tag="xt")
                    nc.sync.dma_start(out=xt[:], in_=xin[i * P:(i + 1) * P, :])
                    tps = psp.tile([P, P], f32, space="PSUM", tag="tps")
                    nc.tensor.transpose(out=tps[:], in_=xt[:], identity=idt[:])
                    xT = pool.tile([P, P], f32, tag="xT")
                    nc.vector.tensor_copy(out=xT[:], in_=tps[:])
                    qps = psp2.tile([P, 3 * HID], f32, space="PSUM", tag="qps")
                    nc.tensor.matmul(out=qps[:], lhsT=xT[:], rhs=wt[:],
                                     start=True, stop=True)
                    ev = pool.tile([P, 3 * HID], f32, tag="ev")
                    nc.vector.tensor_copy(out=ev[:, 0:192], in_=qps[:, 0:192])
                    nc.scalar.activation(out=ev[:, 192:384], in_=qps[:, 192:384],
                                         func=Act.Copy)
                    nc.sync.dma_start(out=qkv_tab[i * P:(i + 1) * P, :], in_=ev[:])

    # ---------------- phase B: edge loop + node update
    def edge_phase(we, ws, xloc, dst_tensor, heads, relu, allgather):
        D = HID // heads
        scale = 1.0 / float(np.sqrt(D))
        W = HID + heads                       # seg-matmul rhs width
        with tile.TileContext(nc) as tc:
            with tc.tile_pool(name="eg_c", bufs=1) as cst, \
                 tc.tile_pool(name="eg_sl", bufs=2) as slp, \
                 tc.tile_pool(name="eg_g", bufs=3) as gp, \
                 tc.tile_pool(name="eg_w", bufs=3) as wp, \
                 tc.tile_pool(name="eg_n", bufs=2) as npool, \
                 tc.tile_pool(name="eg_pe", bufs=2, space="PSUM") as pse, \
                 tc.tile_pool(name="eg_pa", bufs=2, space="PSUM") as psa, \
                 tc.tile_pool(name="eg_pn", bufs=2, space="PSUM") as psn:
                idt = cst.tile([P, P], f32)
                make_identity(nc, idt[:])
                iota_i = cst.tile([P, P], i32)
                nc.gpsimd.iota(iota_i[:], pattern=[[1, P]], base=0,
                               channel_multiplier=0)
                iota_f = cst.tile([P, P], f32)
                nc.vector.tensor_copy(out=iota_f[:], in_=iota_i[:])
                wet = cst.tile([EDGE_DIM, HID], f32)
                nc.sync.dma_start(out=wet[:], in_=we[:])
                wst = cst.tile([HID, HID], f32)
                nc.sync.dma_start(out=wst[:], in_=ws[:])

                nslab = (NCH + SLAB - 1) // SLAB
                slabs = []
                for tl in range(TPC):
                    acc = psa.tile([P, W], f32, space="PSUM", tag="acc")
                    nch_t = int(Cloc[tl])
                    for k in range(nch_t):
                        c = int(off[tl]) + k
                        sb = c // SLAB
                        if sb >= len(slabs):
                            c0 = sb * SLAB
                            c1 = min(NCH, c0 + SLAB)
                            ssl = slp.tile([P, SLAB], i32, tag="ssl")
                            nc.sync.dma_start(out=ssl[:, :c1 - c0], in_=srcT[:, c0:c1])
                            qsl = slp.tile([P, SLAB], i32, tag="qsl")
                            nc.sync.dma_start(out=qsl[:, :c1 - c0], in_=qdstT[:, c0:c1])
                            dsl = slp.tile([P, SLAB], f32, tag="dsl")
                            nc.sync.dma_start(out=dsl[:, :c1 - c0], in_=dstrelT[:, c0:c1])
                            slabs.append((ssl, qsl, dsl))
                        ssl, qsl, dsl = slabs[sb]
                        j = c - sb * SLAB

                        eat = wp.tile([EDGE_DIM, P], f32, tag="eat")
                        nc.sync.dma_start(out=eat[:], in_=eaT[:, c * P:(c + 1) * P])
                        eps = pse.tile([P, HID], f32, space="PSUM", tag="eps")
                        nc.tensor.matmul(out=eps[:], lhsT=eat[:], rhs=wet[:],
                                         start=True, stop=True)

                        kg = gp.tile([P, HID], f32, tag="kg")
                        nc.gpsimd.indirect_dma_start(
                            out=kg[:], out_offset=None, in_=qkv_tab[:],
                            in_offset=bass.IndirectOffsetOnAxis(ap=ssl[:, j:j + 1], axis=0),
                            element_offset=HID)
                        vg = gp.tile([P, HID], f32, tag="vg")
                        nc.gpsimd.indirect_dma_start(
                            out=vg[:], out_offset=None, in_=qkv_tab[:],
                            in_offset=bass.IndirectOffsetOnAxis(ap=ssl[:, j:j + 1], axis=0),
                            element_offset=2 * HID)
                        qg = gp.tile([P, HID], f32, tag="qg")
                        nc.gpsimd.indirect_dma_start(
                            out=qg[:], out_offset=None, in_=qkv_tab[:],
                            in_offset=bass.IndirectOffsetOnAxis(ap=qsl[:, j:j + 1], axis=0),
                            element_offset=0)

                        kj = wp.tile([P, HID], f32, tag="kj")
                        nc.vector.tensor_tensor(out=kj[:], in0=kg[:], in1=eps[:], op=Alu.add)
                        vj = wp.tile([P, HID], f32, tag="vj")
                        nc.vector.tensor_tensor(out=vj[:], in0=vg[:], in1=eps[:], op=Alu.add)

                        rhs = wp.tile([P, W], f32, tag="rhs")
                        if heads == 1:
                            prod = wp.tile([P, HID], f32, tag="prod")
                            alpha = wp.tile([P, 1], f32, tag="alpha")
                            nc.vector.scalar_tensor_tensor(
                                out=prod[:], in0=kj[:], scalar=scale, in1=qg[:],
                                op0=Alu.mult, op1=Alu.mult, accum_out=alpha[:])
                            nc.scalar.activation(out=rhs[:, HID:HID + 1], in_=alpha[:],
                                                 func=Act.Exp)
                            nc.vector.tensor_scalar_mul(
                                out=rhs[:, 0:HID], in0=vj[:], scalar1=rhs[:, HID:HID + 1])
                        else:
                            prod = wp.tile([P, HID], f32, tag="prod")
                            nc.vector.tensor_tensor(out=prod[:], in0=kj[:], in1=qg[:],
                                                    op=Alu.mult)
                            alpha = wp.tile([P, heads], f32, tag="alpha")
                            nc.vector.tensor_reduce(
                                out=alpha[:], in_=prod[:].rearrange("p (h d) -> p h d", h=heads),
                                axis=mybir.AxisListType.X, op=Alu.add)
                            nc.scalar.activation(out=rhs[:, HID:HID + heads], in_=alpha[:],
                                                 func=Act.Exp, scale=scale)
                            nc.vector.tensor_tensor(
                                out=rhs[:, 0:HID], in0=vj[:],
                                in1=rhs[:, HID:HID + heads].to_broadcast([P, heads, D]),
                                op=Alu.mult)

                        S = wp.tile([P, P], f32, tag="S")
                        nc.vector.tensor_tensor(
                            out=S[:], in0=dsl[:, j:j + 1].to_broadcast([P, P]),
                            in1=iota_f[:], op=Alu.is_equal)
                        nc.tensor.matmul(out=acc[:], lhsT=S[:], rhs=rhs[:],
                                         start=(k == 0), stop=(k == nch_t - 1))

                    # ---- node update for tile tl
                    sb_t = npool.tile([P, heads], f32, tag="sb")
                    nc.vector.tensor_scalar_add(out=sb_t[:], in0=acc[:, HID:HID + heads],
                                                scalar1=1e-16)
                    rinv = npool.tile([P, heads], f32, tag="rinv")
                    nc.vector.reciprocal(out=rinv[:], in_=sb_t[:])
                    attn = npool.tile([P, HID], f32, tag="attn")
                    if heads == 1:
                        nc.vector.tensor_scalar_mul(out=attn[:], in0=acc[:, 0:HID],
                                                    scalar1=rinv[:, 0:1])
                    else:
                        nc.vector.tensor_tensor(
                            out=attn[:], in0=acc[:, 0:HID],
                            in1=rinv[:].to_broadcast([P, heads, D]),
                            op=Alu.mult)
                    xt = npool.tile([P, HID], f32, tag="xt")
                    nc.sync.dma_start(out=xt[:], in_=xloc[tl * P:(tl + 1) * P, :])
                    tps = psn.tile([P, P], f32, space="PSUM", tag="tps")
                    nc.tensor.transpose(out=tps[:], in_=xt[:], identity=idt[:])
                    xT = npool.tile([P, P], f32, tag="xT")
                    nc.vector.tensor_copy(out=xT[:], in_=tps[:])
                    sk = psn.tile([P, HID], f32, space="PSUM", tag="sk")
                    nc.tensor.matmul(out=sk[:], lhsT=xT[:], rhs=wst[:],
                                     start=True, stop=True)
                    ht = npool.tile([P, HID], f32, tag="ht")
                    nc.vector.tensor_tensor(out=ht[:], in0=attn[:], in1=sk[:], op=Alu.add)
                    if relu:
                        ht2 = npool.tile([P, HID], f32, tag="ht2")
                        nc.scalar.activation(out=ht2[:], in_=ht[:], func=Act.Lrelu,
                                             alpha=0.01)
                        ht = ht2
                    nc.sync.dma_start(out=dst_tensor[tl * P:(tl + 1) * P, :], in_=ht[:])

                if allgather:
                    nc.gpsimd.collective_compute(
                        "AllGather", Alu.bypass,
                        replica_groups=[list(range(NCORES))],
                        ins=[h_local[:].opt()], outs=[h_full[:].opt()])

    qkv_phase(x_full, Wqkv1)
    edge_phase(We1, Ws1, x_local, h_local, heads=8, relu=True, allgather=True)
    qkv_phase(h_full, Wqkv2)
    edge_phase(We2, Ws2, h_local, out, heads=1, relu=False, allgather=False)

    _legalize_waits(nc)
    return nc


_CACHE = {}


def kernel(x, ei, ea, Wq1, bq1, Wk1, bk1, Wv1, bv1, We1, Ws1, bs1,
           Wq2, bq2, Wk2, bk2, Wv2, bv2, We2, Ws2, bs2):
    from concourse.bass_utils import run_bass_kernel_spmd

    for b in (bq1, bk1, bv1, bs1, bq2, bk2, bv2, bs2):
        assert not np.any(np.asarray(b)), "nonzero biases not supported"

    x = np.asarray(x, np.float32)
    x_pad = np.zeros((NP_, NODE_DIM), np.float32)
    x_pad[:N] = x
    cores, Cloc, off, NCH = _prep(np.asarray(ei), np.asarray(ea))

    key = (NCH, tuple(Cloc))
    if key not in _CACHE:
        _CACHE[key] = _build(Cloc, off, NCH)
    nc = _CACHE[key]

    Wqkv1 = np.ascontiguousarray(np.concatenate(
        [np.asarray(Wq1, np.float32), np.asarray(Wk1, np.float32),
         np.asarray(Wv1, np.float32)], axis=1))
    Wqkv2 = np.ascontiguousarray(np.concatenate(
        [np.asarray(Wq2, np.float32), np.asarray(Wk2, np.float32),
         np.asarray(Wv2, np.float32)], axis=1))

    in_maps = []
    for c in range(NCORES):
        pc = cores[c]
        in_maps.append({
            "x_full": x_pad,
            "x_local": np.ascontiguousarray(x_pad[c * NLC:(c + 1) * NLC]),
            "srcT": pc["srcT"], "qdstT": pc["qdstT"],
            "dstrelT": pc["dstrelT"], "eaT": pc["eaT"],
            "Wqkv1": Wqkv1, "We1": np.asarray(We1, np.float32),
            "Ws1": np.asarray(Ws1, np.float32),
            "Wqkv2": Wqkv2, "We2": np.asarray(We2, np.float32),
            "Ws2": np.asarray(Ws2, np.float32),
        })
    res = run_bass_kernel_spmd(nc, in_maps, list(range(NCORES)))
    global LAST_RESULT
    LAST_RESULT = res
    out = np.concatenate([res.results[c]["out"] for c in range(NCORES)], axis=0)
    return np.ascontiguousarray(out[:N])


LAST_RESULT = None



# revision 9
# speedup vs baseline: 2.0264x; 2.0264x over previous
"""TransformerConv 2-layer GNN encoder on 8 Trainium2 NeuronCores (Bass/Tile).

v2 strategy (graph-partition parallel, bf16 tables, per-tile batching):
  - Nodes padded 50000 -> 50176 = 8 cores x 49 tiles x 128. Each core owns 49
    consecutive node tiles as TARGETS; edges assigned to the dst core, sorted
    by dst, packed into 128-edge chunks per tile (chunk counts equalized
    across cores so the SPMD program is identical).
  - Phase A (per layer): q for LOCAL tiles from x_localT (per-core input,
    SPMD-safe addressing); k|v for ALL tiles from x_fullT -> kv_tab
    [50176, 256] bf16. Host provides x transposed so no PE transposes needed.
  - Edge phase (per layer, per tile, batched over the tile's chunks):
      per chunk: ONE merged k|v indirect gather (512B rows, bf16);
      eps = ea@We on PE (4-chunk PSUM groups, single ACT evacuation);
      S one-hot [slot, c] built batched on DVE; ST = S^T via PE transpose
      (4-chunk PSUM groups); qg = ST^T@qtile on PE (q never gathered);
      batched DVE: kj=k+eps, vj=v+eps, prod=kj*qg, alpha=group-reduce,
      exp on ACT (straight into the rhs tile), vjw=vj*exp;
      segment softmax-sum via S^T@[vjw|exp] accumulated in PSUM per tile;
      fused divide + skip (PE) + lrelu; h stored transposed for layer 2.
  - One AllGather of hT (12.8MB bf16) between the layers.
Softmax: segment-max subtraction skipped (alphas are O(0.3); exact softmax
invariance) and the divide applied after summation - matches reference.
"""
import numpy as np

P = 128
N = 50000
NP_ = 50176
TILES = 392
NCORES = 8
TPC = TILES // NCORES          # 49 tiles per core
NLC = TPC * P                  # 6272 local nodes
NODE_DIM = 128
EDGE_DIM = 16
HID = 128
DSTREL_PAD = 200.0
EGRP = 4                       # chunks per PSUM staging group


# ----------------------------------------------------------------- host prep
def _prep(ei, ea):
    import ml_dtypes
    src = np.asarray(ei[0], dtype=np.int64)
    dst = np.asarray(ei[1], dtype=np.int64)
    ea = np.asarray(ea, dtype=np.float32)

    order = np.argsort(dst, kind="stable")
    src_s, dst_s, ea_s = src[order], dst[order], ea[order]

    tile_of = dst_s // P
    cnt = np.bincount(tile_of, minlength=TILES)
    C = (cnt + P - 1) // P
    Cloc = np.maximum(C.reshape(NCORES, TPC).max(axis=0), 1)   # [TPC]
    NCH = int(Cloc.sum())
    off = np.zeros(TPC, dtype=np.int64)
    off[1:] = np.cumsum(Cloc)[:-1]

    tile_starts = np.searchsorted(tile_of, np.arange(TILES))
    tile_ends = np.searchsorted(tile_of, np.arange(TILES), side="right")
    cores = []
    for c in range(NCORES):
        nslot = NCH * P
        src_sl = np.zeros(nslot, dtype=np.int32)
        drel_sl = np.full(nslot, DSTREL_PAD, dtype=np.float32)
        ea_sl = np.zeros((nslot, EDGE_DIM), dtype=np.float32)
        for tl in range(TPC):
            tg = c * TPC + tl
            a, b = tile_starts[tg], tile_ends[tg]
            if b == a:
                continue
            s0 = off[tl] * P
            src_sl[s0:s0 + b - a] = src_s[a:b]
            drel_sl[s0:s0 + b - a] = (dst_s[a:b] - tg * P).astype(np.float32)
            ea_sl[s0:s0 + b - a] = ea_s[a:b]
        cores.append(dict(
            srcT=np.ascontiguousarray(src_sl.reshape(NCH, P).T),
            dstrelT=np.ascontiguousarray(drel_sl.reshape(NCH, P).T),
            eaT=np.ascontiguousarray(ea_sl.T).astype(ml_dtypes.bfloat16),
        ))
    return cores, Cloc, off, NCH


# ------------------------------------------------------- walrus wait legalize
def _legalize_waits(nc):
    import concourse.mybir as mybir
    k = 0
    for bb in nc.main_func.blocks:
        il = bb.instructions
        new = []
        for ins in il:
            si = ins.sync_info
            if si is not None and len(si.on_wait) > 1:
                waits = list(si.on_wait)
                for w in waits[:-1]:
                    nop = mybir.InstNoOp(name=f"wn{k}-{ins.name}", ins=[], outs=[])
                    k += 1
                    nop.engine = ins.engine
                    nop.sync_info = mybir.SyncInfo(on_wait=[w], on_update=[])
                    new.append(nop)
                ins.sync_info = mybir.SyncInfo(on_wait=[waits[-1]],
                                               on_update=list(si.on_update))
            new.append(ins)
        il[:] = new


# ------------------------------------------------------------- device program
def _build(Cloc, off, NCH):
    import concourse.bass as bass
    import concourse.mybir as mybir
    import concourse.tile as tile
    f32 = mybir.dt.float32
    bf16 = mybir.dt.bfloat16
    i32 = mybir.dt.int32
    Alu = mybir.AluOpType
    Act = mybir.ActivationFunctionType

    nc = bass.Bass()
    dp = nc.declare_dram_parameter
    x_fullT = dp("x_fullT", [NODE_DIM, NP_], bf16, isOutput=False)
    x_localT = dp("x_localT", [NODE_DIM, NLC], bf16, isOutput=False)
    srcT = dp("srcT", [P, NCH], i32, isOutput=False)
    dstrelT = dp("dstrelT", [P, NCH], f32, isOutput=False)
    eaT = dp("eaT", [EDGE_DIM, NCH * P], bf16, isOutput=False)
    iota_in = dp("iota_in", [P, P], f32, isOutput=False)
    ident_in = dp("ident_in", [P, P], bf16, isOutput=False)
    Wqkv1 = dp("Wqkv1", [NODE_DIM, 3 * HID], bf16, isOutput=False)
    We1 = dp("We1", [EDGE_DIM, HID], bf16, isOutput=False)
    Ws1 = dp("Ws1", [NODE_DIM, HID], bf16, isOutput=False)
    Wqkv2 = dp("Wqkv2", [HID, 3 * HID], bf16, isOutput=False)
    We2 = dp("We2", [EDGE_DIM, HID], bf16, isOutput=False)
    Ws2 = dp("Ws2", [HID, HID], bf16, isOutput=False)
    out = dp("out", [NLC, HID], f32, isOutput=True)

    kv_tab = nc.dram_tensor("kv_tab", [NP_, 2 * HID], bf16, kind="Internal")
    q_loc = nc.dram_tensor("q_loc", [NLC, HID], bf16, kind="Internal")
    hT_loc = nc.dram_tensor("hT_loc", [HID, NLC], bf16, kind="Internal")
    hT_full = nc.dram_tensor("hT_full", [NCORES * HID, NLC], bf16, kind="Internal")

    # ---------------- phase A: q for local tiles, k|v table for all tiles
    def qkv_phase(locT, fullT, wqkv):
        with tile.TileContext(nc) as tc:
            with tc.tile_pool(name="qa_c", bufs=1) as cst, \
                 tc.tile_pool(name="qa_s", bufs=3) as pool, \
                 tc.tile_pool(name="qa_p", bufs=3, space="PSUM") as psp:
                wt = cst.tile([NODE_DIM, 3 * HID], bf16)
                nc.sync.dma_start(out=wt[:], in_=wqkv[:])
                # local q (49 tiles)
                for i in range(TPC):
                    xt = pool.tile([P, P], bf16, tag="xl")
                    nc.sync.dma_start(out=xt[:], in_=locT[:, i * P:(i + 1) * P])
                    ps = psp.tile([P, HID], f32, space="PSUM", tag="pq")
                    nc.tensor.matmul(out=ps[:], lhsT=xt[:], rhs=wt[:, 0:HID],
                                     start=True, stop=True)
                    ev = pool.tile([P, HID], bf16, tag="evq")
                    nc.scalar.activation(out=ev[:], in_=ps[:], func=Act.Copy)
                    nc.sync.dma_start(out=q_loc[i * P:(i + 1) * P, :], in_=ev[:])
                # full k|v (392 tiles)
                for i in range(TILES):
                    xt = pool.tile([P, P], bf16, tag="xf")
                    nc.sync.dma_start(out=xt[:], in_=fullT[:, i * P:(i + 1) * P])
                    ps = psp.tile([P, 2 * HID], f32, space="PSUM", tag="pkv")
                    nc.tensor.matmul(out=ps[:], lhsT=xt[:], rhs=wt[:, HID:3 * HID],
                                     start=True, stop=True)
                    ev = pool.tile([P, 2 * HID], bf16, tag="evkv")
                    nc.scalar.activation(out=ev[:], in_=ps[:], func=Act.Copy)
                    nc.sync.dma_start(out=kv_tab[i * P:(i + 1) * P, :], in_=ev[:])

    # ---------------- edge phase
    def edge_phase(we, ws, xlocT, heads, relu, allgather):
        D = HID // heads
        scale = 1.0 / float(np.sqrt(D))
        W = HID + heads
        NMAX = int(Cloc.max())
        with tile.TileContext(nc) as tc:
            with tc.tile_pool(name="eg_c", bufs=1) as cst, \
                 tc.tile_pool(name="eg_sl", bufs=2) as slp, \
                 tc.tile_pool(name="eg_g", bufs=2) as gp, \
                 tc.tile_pool(name="eg_w", bufs=2) as wp, \
                 tc.tile_pool(name="eg_n", bufs=2) as npool, \
                 tc.tile_pool(name="eg_ps", bufs=3, space="PSUM") as pstage, \
                 tc.tile_pool(name="eg_pt", bufs=2, space="PSUM") as pst, \
                 tc.tile_pool(name="eg_pa", bufs=2, space="PSUM") as psacc:
                iota_f = cst.tile([P, P], f32)
                nc.sync.dma_start(out=iota_f[:], in_=iota_in[:])
                idt = cst.tile([P, P], bf16)
                nc.sync.dma_start(out=idt[:], in_=ident_in[:])
                wet = cst.tile([EDGE_DIM, HID], bf16)
                nc.sync.dma_start(out=wet[:], in_=we[:])
                wst = cst.tile([HID, HID], bf16)
                nc.sync.dma_start(out=wst[:], in_=ws[:])

                for tl in range(TPC):
                    nch = int(Cloc[tl])
                    c0 = int(off[tl])
                    FD = nch * P
                    ngr = (nch + EGRP - 1) // EGRP

                    ssl = slp.tile([P, NMAX], i32, tag="ssl")
                    nc.sync.dma_start(out=ssl[:, 0:nch], in_=srcT[:, c0:c0 + nch])
                    dsl = slp.tile([P, NMAX], f32, tag="dsl")
                    nc.sync.dma_start(out=dsl[:, 0:nch], in_=dstrelT[:, c0:c0 + nch])
                    ea_all = slp.tile([EDGE_DIM, NMAX * P], bf16, tag="ea")
                    nc.sync.dma_start(out=ea_all[:, 0:FD],
                                      in_=eaT[:, c0 * P:(c0 + nch) * P])
                    qtile = slp.tile([P, HID], bf16, tag="qt")
                    nc.sync.dma_start(out=qtile[:],
                                      in_=q_loc[tl * P:(tl + 1) * P, :])
                    xsk = slp.tile([P, P], bf16, tag="xsk")
                    nc.sync.dma_start(out=xsk[:], in_=xlocT[:, tl * P:(tl + 1) * P])

                    # merged k|v gathers, one per chunk
                    kvg = gp.tile([P, NMAX * 2 * HID], bf16, tag="kvg")
                    for k in range(nch):
                        nc.gpsimd.indirect_dma_start(
                            out=kvg[:, k * 2 * HID:(k + 1) * 2 * HID],
                            out_offset=None, in_=kv_tab[:],
                            in_offset=bass.IndirectOffsetOnAxis(
                                ap=ssl[:, k:k + 1], axis=0))

                    # S one-hot [slot, (j, c)] batched
                    S_all = wp.tile([P, NMAX * P], bf16, tag="S")
                    nc.vector.tensor_tensor(
                        out=S_all[:, 0:FD].rearrange("p (j c) -> p j c", j=nch),
                        in0=dsl[:, 0:nch].unsqueeze(2).to_broadcast([P, nch, P]),
                        in1=iota_f[:].unsqueeze(1).to_broadcast([P, nch, P]),
                        op=Alu.is_equal)

                    # eps / ST / qg staged through PSUM in EGRP-chunk groups
                    eps_sb = wp.tile([P, NMAX * HID], bf16, tag="eps")
                    st_sb = wp.tile([P, NMAX * P], bf16, tag="st")
                    qg_sb = wp.tile([P, NMAX * HID], bf16, tag="qg")
                    for g in range(ngr):
                        k0, k1 = g * EGRP, min(nch, (g + 1) * EGRP)
                        nk = k1 - k0
                        pe = pstage.tile([P, EGRP * HID], f32, space="PSUM",
                                         tag="stage")
                        for k in range(k0, k1):
                            j = k - k0
                            nc.tensor.matmul(
                                out=pe[:, j * HID:(j + 1) * HID],
                                lhsT=ea_all[:, k * P:(k + 1) * P],
                                rhs=wet[:], start=True, stop=True)
                        nc.scalar.activation(out=eps_sb[:, k0 * HID:k1 * HID],
                                             in_=pe[:, 0:nk * HID], func=Act.Copy)
                        pt = pst.tile([P, EGRP * P], bf16, space="PSUM",
                                      tag="staget")
                        for k in range(k0, k1):
                            j = k - k0
                            nc.tensor.transpose(
                                out=pt[:, j * P:(j + 1) * P],
                                in_=S_all[:, k * P:(k + 1) * P], identity=idt[:])
                        nc.scalar.activation(out=st_sb[:, k0 * P:k1 * P],
                                             in_=pt[:, 0:nk * P], func=Act.Copy)
                        pq = pstage.tile([P, EGRP * HID], f32, space="PSUM",
                                         tag="stage")
                        for k in range(k0, k1):
                            j = k - k0
                            nc.tensor.matmul(
                                out=pq[:, j * HID:(j + 1) * HID],
                                lhsT=st_sb[:, k * P:(k + 1) * P],
                                rhs=qtile[:], start=True, stop=True)
                        nc.scalar.activation(out=qg_sb[:, k0 * HID:k1 * HID],
                                             in_=pq[:, 0:nk * HID], func=Act.Copy)

                    # batched DVE: kj, vj, prod, alpha
                    kj = wp.tile([P, NMAX * HID], bf16, tag="kj")
                    nc.vector.tensor_tensor(
                        out=kj[:, 0:FD].rearrange("p (j d) -> p j d", j=nch),
                        in0=kvg[:, 0:nch * 2 * HID].rearrange(
                            "p (j d) -> p j d", j=nch)[:, :, 0:HID],
                        in1=eps_sb[:, 0:FD].rearrange("p (j d) -> p j d", j=nch),
                        op=Alu.add)
                    vj = wp.tile([P, NMAX * HID], bf16, tag="vj")
                    nc.vector.tensor_tensor(
                        out=vj[:, 0:FD].rearrange("p (j d) -> p j d", j=nch),
                        in0=kvg[:, 0:nch * 2 * HID].rearrange(
                            "p (j d) -> p j d", j=nch)[:, :, HID:2 * HID],
                        in1=eps_sb[:, 0:FD].rearrange("p (j d) -> p j d", j=nch),
                        op=Alu.add)
                    prod = wp.tile([P, NMAX * HID], bf16, tag="prod")
                    nc.vector.tensor_tensor(out=prod[:, 0:FD], in0=kj[:, 0:FD],
                                            in1=qg_sb[:, 0:FD], op=Alu.mult)
                    alpha = wp.tile([P, NMAX * 8], f32, tag="alpha")
                    nc.vector.tensor_reduce(
                        out=alpha[:, 0:nch * heads],
                        in_=prod[:, 0:FD].rearrange("p (g d) -> p g d", d=D),
                        axis=mybir.AxisListType.X, op=Alu.add)

                    # rhs = [vj*exp | exp]
                    rhs = wp.tile([P, NMAX * W], bf16, tag="rhs")
                    rhs3 = rhs[:, 0:nch * W].rearrange("p (j w) -> p j w", j=nch)
                    nc.scalar.activation(
                        out=rhs3[:, :, HID:W],
                        in_=alpha[:, 0:nch * heads].rearrange(
                            "p (j h) -> p j h", j=nch),
                        func=Act.Exp, scale=scale)
                    nc.vector.tensor_tensor(
                        out=rhs3[:, :, 0:HID].rearrange(
                            "p j (h d) -> p j h d", h=heads),
                        in0=vj[:, 0:FD].rearrange(
                            "p (j h d) -> p j h d", j=nch, h=heads),
                        in1=rhs3[:, :, HID:W].unsqueeze(3).to_broadcast(
                            [P, nch, heads, D]),
                        op=Alu.mult)

                    # segment sum via one-hot matmul, accumulated per tile
                    acc = psacc.tile([P, W], f32, space="PSUM", tag="acc")
                    for k in range(nch):
                        nc.tensor.matmul(
                            out=acc[:], lhsT=S_all[:, k * P:(k + 1) * P],
                            rhs=rhs[:, k * W:(k + 1) * W],
                            start=(k == 0), stop=(k == nch - 1))

                    # ---- node update
                    sb_t = npool.tile([P, heads], f32, tag="sb")
                    nc.vector.tensor_scalar_add(out=sb_t[:],
                                                in0=acc[:, HID:W], scalar1=1e-16)
                    rinv = npool.tile([P, heads], f32, tag="rinv")
                    nc.vector.reciprocal(out=rinv[:], in_=sb_t[:])
                    attn = npool.tile([P, HID], f32, tag="attn")
                    nc.vector.tensor_tensor(
                        out=attn[:].rearrange("p (h d) -> p h d", h=heads),
                        in0=acc[:, 0:HID].rearrange("p (h d) -> p h d", h=heads),
                        in1=rinv[:].unsqueeze(2).to_broadcast([P, heads, D]),
                        op=Alu.mult)
                    skt = pstage.tile([P, EGRP * HID], f32, space="PSUM",
                                      tag="stage")
                    sk = skt[:, 0:HID]
                    nc.tensor.matmul(out=sk, lhsT=xsk[:], rhs=wst[:],
                                     start=True, stop=True)
                    ht = npool.tile([P, HID], f32, tag="ht")
                    nc.vector.tensor_tensor(out=ht[:], in0=attn[:], in1=sk,
                                            op=Alu.add)
                    if relu:
                        ht2 = npool.tile([P, HID], bf16, tag="ht2")
                        nc.scalar.activation(out=ht2[:], in_=ht[:], func=Act.Lrelu,
                                             alpha=0.01)
                        tpt = pst.tile([P, EGRP * P], bf16, space="PSUM",
                                       tag="staget")
                        tp = tpt[:, 0:P]
                        nc.tensor.transpose(out=tp, in_=ht2[:], identity=idt[:])
                        hTt = npool.tile([P, P], bf16, tag="hTt")
                        nc.scalar.activation(out=hTt[:], in_=tp, func=Act.Copy)
                        nc.sync.dma_start(out=hT_loc[:, tl * P:(tl + 1) * P],
                                          in_=hTt[:])
                    else:
                        nc.sync.dma_start(out=out[tl * P:(tl + 1) * P, :],
                                          in_=ht[:])

                if allgather:
                    nc.gpsimd.collective_compute(
                        "AllGather", Alu.bypass,
                        replica_groups=[list(range(NCORES))],
                        ins=[hT_loc[:].opt()], outs=[hT_full[:].opt()])

    # layer-2 phase A reads the gathered hT (full) and local hT
    def qkv_phase2():
        with tile.TileContext(nc) as tc:
            with tc.tile_pool(name="qb_c", bufs=1) as cst, \
                 tc.tile_pool(name="qb_s", bufs=3) as pool, \
                 tc.tile_pool(name="qb_p", bufs=3, space="PSUM") as psp:
                wt = cst.tile([HID, 3 * HID], bf16)
                nc.sync.dma_start(out=wt[:], in_=Wqkv2[:])
                for i in range(TPC):
                    xt = pool.tile([P, P], bf16, tag="xl")
                    nc.sync.dma_start(out=xt[:], in_=hT_loc[:, i * P:(i + 1) * P])
                    ps = psp.tile([P, HID], f32, space="PSUM", tag="pq")
                    nc.tensor.matmul(out=ps[:], lhsT=xt[:], rhs=wt[:, 0:HID],
                                     start=True, stop=True)
                    ev = pool.tile([P, HID], bf16, tag="evq")
                    nc.scalar.activation(out=ev[:], in_=ps[:], func=Act.Copy)
                    nc.sync.dma_start(out=q_loc[i * P:(i + 1) * P, :], in_=ev[:])
                for i in range(TILES):
                    ci, lt = divmod(i, TPC)
                    xt = pool.tile([P, P], bf16, tag="xf")
                    nc.sync.dma_start(
                        out=xt[:],
                        in_=hT_full[ci * HID:(ci + 1) * HID,
                                    lt * P:(lt + 1) * P])
                    ps = psp.tile([P, 2 * HID], f32, space="PSUM", tag="pkv")
                    nc.tensor.matmul(out=ps[:], lhsT=xt[:], rhs=wt[:, HID:3 * HID],
                                     start=True, stop=True)
                    ev = pool.tile([P, 2 * HID], bf16, tag="evkv")
                    nc.scalar.activation(out=ev[:], in_=ps[:], func=Act.Copy)
                    nc.sync.dma_start(out=kv_tab[i * P:(i + 1) * P, :], in_=ev[:])

    qkv_phase(x_localT, x_fullT, Wqkv1)
    edge_phase(We1, Ws1, x_localT, heads=8, relu=True, allgather=True)
    qkv_phase2()
    edge_phase(We2, Ws2, hT_loc, heads=1, relu=False, allgather=False)

    _legalize_waits(nc)
    return nc


_CACHE = {}


def kernel(x, ei, ea, Wq1, bq1, Wk1, bk1, Wv1, bv1, We1, Ws1, bs1,
           Wq2, bq2, Wk2, bk2, Wv2, bv2, We2, Ws2, bs2):
    import ml_dtypes
    from concourse.bass_utils import run_bass_kernel_spmd
    bf = ml_dtypes.bfloat16

    for b in (bq1, bk1, bv1, bs1, bq2, bk2, bv2, bs2):
        assert not np.any(np.asarray(b)), "nonzero biases not supported"

    x = np.asarray(x, np.float32)
    x_pad = np.zeros((NP_, NODE_DIM), np.float32)
    x_pad[:N] = x
    x_fullT = np.ascontiguousarray(x_pad.T).astype(bf)
    cores, Cloc, off, NCH = _prep(np.asarray(ei), np.asarray(ea))

    key = (NCH, tuple(Cloc))
    if key not in _CACHE:
        _CACHE[key] = _build(Cloc, off, NCH)
    nc = _CACHE[key]

    def cat3(a, b, c):
        return np.ascontiguousarray(np.concatenate(
            [np.asarray(a, np.float32), np.asarray(b, np.float32),
             np.asarray(c, np.float32)], axis=1)).astype(bf)

    Wqkv1 = cat3(Wq1, Wk1, Wv1)
    Wqkv2 = cat3(Wq2, Wk2, Wv2)
    iota_in = np.tile(np.arange(P, dtype=np.float32)[None, :], (P, 1))
    ident_in = np.eye(P, dtype=np.float32).astype(bf)

    in_maps = []
    for c in range(NCORES):
        pc = cores[c]
        in_maps.append({
            "x_fullT": x_fullT,
            "x_localT": np.ascontiguousarray(
                x_fullT[:, c * NLC:(c + 1) * NLC]),
            "srcT": pc["srcT"], "dstrelT": pc["dstrelT"], "eaT": pc["eaT"],
            "iota_in": iota_in, "ident_in": ident_in,
            "Wqkv1": Wqkv1, "We1": np.asarray(We1, np.float32).astype(bf),
            "Ws1": np.asarray(Ws1, np.float32).astype(bf),
            "Wqkv2": Wqkv2, "We2": np.asarray(We2, np.float32).astype(bf),
            "Ws2": np.asarray(Ws2, np.float32).astype(bf),
        })
    res = run_bass_kernel_spmd(nc, in_maps, list(range(NCORES)))
    global LAST_RESULT
    LAST_RESULT = res
    out = np.concatenate([res.results[c]["out"] for c in range(NCORES)], axis=0)
    return np.ascontiguousarray(out[:N])


LAST_RESULT = None


# revision 12
# speedup vs baseline: 2.5297x; 1.2484x over previous
"""TransformerConv 2-layer GNN encoder on 8 Trainium2 NeuronCores (Bass/Tile).

v2 strategy (graph-partition parallel, bf16 tables, per-tile batching):
  - Nodes padded 50000 -> 50176 = 8 cores x 49 tiles x 128. Each core owns 49
    consecutive node tiles as TARGETS; edges assigned to the dst core, sorted
    by dst, packed into 128-edge chunks per tile (chunk counts equalized
    across cores so the SPMD program is identical).
  - Phase A (per layer): q for LOCAL tiles from x_localT (per-core input,
    SPMD-safe addressing); k|v for ALL tiles from x_fullT -> kv_tab
    [50176, 256] bf16. Host provides x transposed so no PE transposes needed.
  - Edge phase (per layer, per tile, batched over the tile's chunks):
      per chunk: ONE merged k|v indirect gather (512B rows, bf16);
      eps = ea@We on PE (4-chunk PSUM groups, single ACT evacuation);
      S one-hot [slot, c] built batched on DVE; ST = S^T via PE transpose
      (4-chunk PSUM groups); qg = ST^T@qtile on PE (q never gathered);
      batched DVE: kj=k+eps, vj=v+eps, prod=kj*qg, alpha=group-reduce,
      exp on ACT (straight into the rhs tile), vjw=vj*exp;
      segment softmax-sum via S^T@[vjw|exp] accumulated in PSUM per tile;
      fused divide + skip (PE) + lrelu; h stored transposed for layer 2.
  - One AllGather of hT (12.8MB bf16) between the layers.
Softmax: segment-max subtraction skipped (alphas are O(0.3); exact softmax
invariance) and the divide applied after summation - matches reference.
"""
import numpy as np

P = 128
N = 50000
NP_ = 50176
TILES = 392
NCORES = 8
TPC = TILES // NCORES          # 49 tiles per core
NLC = TPC * P                  # 6272 local nodes
NODE_DIM = 128
EDGE_DIM = 16
HID = 128
DSTREL_PAD = 200.0
EGRP = 4                       # chunks per PSUM staging group


# ----------------------------------------------------------------- host prep
def _prep(ei, ea):
    import ml_dtypes
    src = np.asarray(ei[0], dtype=np.int64)
    dst = np.asarray(ei[1], dtype=np.int64)
    ea = np.asarray(ea, dtype=np.float32)

    order = np.argsort(dst, kind="stable")
    src_s, dst_s, ea_s = src[order], dst[order], ea[order]

    tile_of = dst_s // P
    cnt = np.bincount(tile_of, minlength=TILES)
    C = (cnt + P - 1) // P
    Cloc = np.maximum(C.reshape(NCORES, TPC).max(axis=0), 1)   # [TPC]
    NCH = int(Cloc.sum())
    off = np.zeros(TPC, dtype=np.int64)
    off[1:] = np.cumsum(Cloc)[:-1]

    tile_starts = np.searchsorted(tile_of, np.arange(TILES))
    tile_ends = np.searchsorted(tile_of, np.arange(TILES), side="right")
    cores = []
    for c in range(NCORES):
        nslot = NCH * P
        src_sl = np.zeros(nslot, dtype=np.int32)
        drel_sl = np.full(nslot, DSTREL_PAD, dtype=np.float32)
        ea_sl = np.zeros((nslot, EDGE_DIM), dtype=np.float32)
        for tl in range(TPC):
            tg = c * TPC + tl
            a, b = tile_starts[tg], tile_ends[tg]
            if b == a:
                continue
            s0 = off[tl] * P
            src_sl[s0:s0 + b - a] = src_s[a:b]
            drel_sl[s0:s0 + b - a] = (dst_s[a:b] - tg * P).astype(np.float32)
            ea_sl[s0:s0 + b - a] = ea_s[a:b]
        cores.append(dict(
            srcT=np.ascontiguousarray(src_sl.reshape(NCH, P).T),
            dstrelT=np.ascontiguousarray(drel_sl.reshape(NCH, P).T),
            eaT=np.ascontiguousarray(ea_sl.T).astype(ml_dtypes.bfloat16),
        ))
    return cores, Cloc, off, NCH


# ------------------------------------------------------- walrus wait legalize
def _legalize_waits(nc):
    import concourse.mybir as mybir
    k = 0
    for bb in nc.main_func.blocks:
        il = bb.instructions
        new = []
        for ins in il:
            si = ins.sync_info
            if si is not None and len(si.on_wait) > 1:
                waits = list(si.on_wait)
                for w in waits[:-1]:
                    nop = mybir.InstNoOp(name=f"wn{k}-{ins.name}", ins=[], outs=[])
                    k += 1
                    nop.engine = ins.engine
                    nop.sync_info = mybir.SyncInfo(on_wait=[w], on_update=[])
                    new.append(nop)
                ins.sync_info = mybir.SyncInfo(on_wait=[waits[-1]],
                                               on_update=list(si.on_update))
            new.append(ins)
        il[:] = new


# ------------------------------------------------------------- device program
def _build(Cloc, off, NCH):
    import concourse.bass as bass
    import concourse.mybir as mybir
    import concourse.tile as tile
    f32 = mybir.dt.float32
    bf16 = mybir.dt.bfloat16
    i32 = mybir.dt.int32
    Alu = mybir.AluOpType
    Act = mybir.ActivationFunctionType

    nc = bass.Bass()
    dp = nc.declare_dram_parameter
    x_fullT = dp("x_fullT", [NODE_DIM, NP_], bf16, isOutput=False)
    x_localT = dp("x_localT", [NODE_DIM, NLC], bf16, isOutput=False)
    srcT = dp("srcT", [P, NCH], i32, isOutput=False)
    dstrelT = dp("dstrelT", [P, NCH], f32, isOutput=False)
    eaT = dp("eaT", [EDGE_DIM, NCH * P], bf16, isOutput=False)
    iota_in = dp("iota_in", [P, P], f32, isOutput=False)
    ident_in = dp("ident_in", [P, P], bf16, isOutput=False)
    Wqkv1 = dp("Wqkv1", [NODE_DIM, 3 * HID], bf16, isOutput=False)
    We1 = dp("We1", [EDGE_DIM, HID], bf16, isOutput=False)
    Ws1 = dp("Ws1", [NODE_DIM, HID], bf16, isOutput=False)
    Wqkv2 = dp("Wqkv2", [HID, 3 * HID], bf16, isOutput=False)
    We2 = dp("We2", [EDGE_DIM, HID], bf16, isOutput=False)
    Ws2 = dp("Ws2", [HID, HID], bf16, isOutput=False)
    out = dp("out", [NLC, HID], f32, isOutput=True)

    kv_tab = nc.dram_tensor("kv_tab", [NP_, 2 * HID], bf16, kind="Internal")
    q_loc = nc.dram_tensor("q_loc", [NLC, HID], bf16, kind="Internal")
    hT_loc = nc.dram_tensor("hT_loc", [HID, NLC], bf16, kind="Internal")
    hT_full = nc.dram_tensor("hT_full", [NCORES * HID, NLC], bf16, kind="Internal")

    # ---------------- phase A: q for local tiles, k|v table for all tiles
    # 4-tile batches: one load DMA, 4 matmuls, one evacuation, one store DMA.
    BA = 4

    def _qkv_loops(tc, cst, pool, psp, wt, qsrcT, kvsrc_slice):
        for i0 in range(0, TPC, BA):
            nb = min(BA, TPC - i0)
            xt = pool.tile([P, BA * P], bf16, tag="xl")
            nc.sync.dma_start(out=xt[:, 0:nb * P],
                              in_=qsrcT(i0, nb))
            ps = psp.tile([P, BA * HID], f32, space="PSUM", tag="pq")
            for j in range(nb):
                nc.tensor.matmul(out=ps[:, j * HID:(j + 1) * HID],
                                 lhsT=xt[:, j * P:(j + 1) * P],
                                 rhs=wt[:, 0:HID], start=True, stop=True)
            ev = pool.tile([P, BA * HID], bf16, tag="evq")
            nc.scalar.activation(out=ev[:, 0:nb * HID], in_=ps[:, 0:nb * HID],
                                 func=Act.Copy)
            nc.sync.dma_start(
                out=q_loc[i0 * P:(i0 + nb) * P, :].rearrange(
                    "(j p) d -> p j d", j=nb),
                in_=ev[:, 0:nb * HID].rearrange("p (j d) -> p j d", j=nb))
        for i0 in range(0, TILES, BA):
            nb = min(BA, TILES - i0)
            xt = pool.tile([P, BA * P], bf16, tag="xf")
            nc.sync.dma_start(out=xt[:, 0:nb * P], in_=kvsrc_slice(i0, nb))
            ps = psp.tile([P, BA * 2 * HID], f32, space="PSUM", tag="pkv")
            for j in range(nb):
                nc.tensor.matmul(out=ps[:, j * 2 * HID:(j + 1) * 2 * HID],
                                 lhsT=xt[:, j * P:(j + 1) * P],
                                 rhs=wt[:, HID:3 * HID], start=True, stop=True)
            ev = pool.tile([P, BA * 2 * HID], bf16, tag="evkv")
            nc.scalar.activation(out=ev[:, 0:nb * 2 * HID],
                                 in_=ps[:, 0:nb * 2 * HID], func=Act.Copy)
            nc.sync.dma_start(
                out=kv_tab[i0 * P:(i0 + nb) * P, :].rearrange(
                    "(j p) d -> p j d", j=nb),
                in_=ev[:, 0:nb * 2 * HID].rearrange("p (j d) -> p j d", j=nb))

    def qkv_phase(locT, fullT, wqkv):
        with tile.TileContext(nc) as tc:
            with tc.tile_pool(name="qa_c", bufs=1) as cst, \
                 tc.tile_pool(name="qa_s", bufs=3) as pool, \
                 tc.tile_pool(name="qa_p", bufs=2, space="PSUM") as psp:
                wt = cst.tile([NODE_DIM, 3 * HID], bf16)
                nc.sync.dma_start(out=wt[:], in_=wqkv[:])
                _qkv_loops(tc, cst, pool, psp, wt,
                           lambda i0, nb: locT[:, i0 * P:(i0 + nb) * P],
                           lambda i0, nb: fullT[:, i0 * P:(i0 + nb) * P])

    # ---------------- edge phase
    def edge_phase(we, ws, xlocT, heads, relu, allgather):
        D = HID // heads
        scale = 1.0 / float(np.sqrt(D))
        W = HID + heads
        NMAX = int(Cloc.max())
        with tile.TileContext(nc) as tc:
            with tc.tile_pool(name="eg_c", bufs=1) as cst, \
                 tc.tile_pool(name="eg_sl", bufs=3) as slp, \
                 tc.tile_pool(name="eg_g", bufs=3) as gp, \
                 tc.tile_pool(name="eg_w", bufs=2) as wp, \
                 tc.tile_pool(name="eg_n", bufs=2) as npool, \
                 tc.tile_pool(name="eg_ps", bufs=3, space="PSUM") as pstage, \
                 tc.tile_pool(name="eg_pt", bufs=2, space="PSUM") as pst, \
                 tc.tile_pool(name="eg_pa", bufs=2, space="PSUM") as psacc:
                iota_f = cst.tile([P, P], f32)
                nc.sync.dma_start(out=iota_f[:], in_=iota_in[:])
                idt = cst.tile([P, P], bf16)
                nc.sync.dma_start(out=idt[:], in_=ident_in[:])
                wet = cst.tile([EDGE_DIM, HID], bf16)
                nc.sync.dma_start(out=wet[:], in_=we[:])
                wst = cst.tile([HID, HID], bf16)
                nc.sync.dma_start(out=wst[:], in_=ws[:])

                for tl in range(TPC):
                    nch = int(Cloc[tl])
                    c0 = int(off[tl])
                    FD = nch * P
                    ngr = (nch + EGRP - 1) // EGRP

                    ssl = slp.tile([P, NMAX], i32, tag="ssl")
                    nc.sync.dma_start(out=ssl[:, 0:nch], in_=srcT[:, c0:c0 + nch])
                    dsl = slp.tile([P, NMAX], f32, tag="dsl")
                    nc.sync.dma_start(out=dsl[:, 0:nch], in_=dstrelT[:, c0:c0 + nch])
                    ea_all = slp.tile([EDGE_DIM, NMAX * P], bf16, tag="ea")
                    nc.sync.dma_start(out=ea_all[:, 0:FD],
                                      in_=eaT[:, c0 * P:(c0 + nch) * P])
                    qtile = slp.tile([P, HID], bf16, tag="qt")
                    nc.sync.dma_start(out=qtile[:],
                                      in_=q_loc[tl * P:(tl + 1) * P, :])
                    xsk = slp.tile([P, P], bf16, tag="xsk")
                    nc.sync.dma_start(out=xsk[:], in_=xlocT[:, tl * P:(tl + 1) * P])

                    # merged k|v gathers, one per chunk
                    kvg = gp.tile([P, NMAX * 2 * HID], bf16, tag="kvg")
                    for k in range(nch):
                        nc.gpsimd.indirect_dma_start(
                            out=kvg[:, k * 2 * HID:(k + 1) * 2 * HID],
                            out_offset=None, in_=kv_tab[:],
                            in_offset=bass.IndirectOffsetOnAxis(
                                ap=ssl[:, k:k + 1], axis=0))

                    # S one-hot [slot, (j, c)] batched
                    S_all = wp.tile([P, NMAX * P], bf16, tag="S")
                    nc.vector.tensor_tensor(
                        out=S_all[:, 0:FD].rearrange("p (j c) -> p j c", j=nch),
                        in0=dsl[:, 0:nch].unsqueeze(2).to_broadcast([P, nch, P]),
                        in1=iota_f[:].unsqueeze(1).to_broadcast([P, nch, P]),
                        op=Alu.is_equal)

                    # eps / ST / qg staged through PSUM in EGRP-chunk groups
                    eps_sb = wp.tile([P, NMAX * HID], bf16, tag="eps")
                    st_sb = wp.tile([P, NMAX * P], bf16, tag="st")
                    qg_sb = wp.tile([P, NMAX * HID], bf16, tag="qg")
                    for g in range(ngr):
                        k0, k1 = g * EGRP, min(nch, (g + 1) * EGRP)
                        nk = k1 - k0
                        pe = pstage.tile([P, EGRP * HID], f32, space="PSUM",
                                         tag="stage")
                        for k in range(k0, k1):
                            j = k - k0
                            nc.tensor.matmul(
                                out=pe[:, j * HID:(j + 1) * HID],
                                lhsT=ea_all[:, k * P:(k + 1) * P],
                                rhs=wet[:], start=True, stop=True)
                        nc.scalar.activation(out=eps_sb[:, k0 * HID:k1 * HID],
                                             in_=pe[:, 0:nk * HID], func=Act.Copy)
                        pt = pst.tile([P, EGRP * P], bf16, space="PSUM",
                                      tag="staget")
                        for k in range(k0, k1):
                            j = k - k0
                            nc.tensor.transpose(
                                out=pt[:, j * P:(j + 1) * P],
                                in_=S_all[:, k * P:(k + 1) * P], identity=idt[:])
                        nc.scalar.activation(out=st_sb[:, k0 * P:k1 * P],
                                             in_=pt[:, 0:nk * P], func=Act.Copy)
                        pq = pstage.tile([P, EGRP * HID], f32, space="PSUM",
                                         tag="stage")
                        for k in range(k0, k1):
                            j = k - k0
                            nc.tensor.matmul(
                                out=pq[:, j * HID:(j + 1) * HID],
                                lhsT=st_sb[:, k * P:(k + 1) * P],
                                rhs=qtile[:], start=True, stop=True)
                        nc.scalar.activation(out=qg_sb[:, k0 * HID:k1 * HID],
                                             in_=pq[:, 0:nk * HID], func=Act.Copy)

                    # batched DVE: kj, vj, prod, alpha
                    kj = wp.tile([P, NMAX * HID], bf16, tag="kj")
                    nc.vector.tensor_tensor(
                        out=kj[:, 0:FD].rearrange("p (j d) -> p j d", j=nch),
                        in0=kvg[:, 0:nch * 2 * HID].rearrange(
                            "p (j d) -> p j d", j=nch)[:, :, 0:HID],
                        in1=eps_sb[:, 0:FD].rearrange("p (j d) -> p j d", j=nch),
                        op=Alu.add)
                    vj = wp.tile([P, NMAX * HID], bf16, tag="vj")
                    nc.vector.tensor_tensor(
                        out=vj[:, 0:FD].rearrange("p (j d) -> p j d", j=nch),
                        in0=kvg[:, 0:nch * 2 * HID].rearrange(
                            "p (j d) -> p j d", j=nch)[:, :, HID:2 * HID],
                        in1=eps_sb[:, 0:FD].rearrange("p (j d) -> p j d", j=nch),
                        op=Alu.add)
                    prod = wp.tile([P, NMAX * HID], bf16, tag="prod")
                    nc.vector.tensor_tensor(out=prod[:, 0:FD], in0=kj[:, 0:FD],
                                            in1=qg_sb[:, 0:FD], op=Alu.mult)
                    alpha = wp.tile([P, NMAX * 8], f32, tag="alpha")
                    nc.vector.tensor_reduce(
                        out=alpha[:, 0:nch * heads],
                        in_=prod[:, 0:FD].rearrange("p (g d) -> p g d", d=D),
                        axis=mybir.AxisListType.X, op=Alu.add)

                    # rhs = [vj*exp | exp]
                    rhs = wp.tile([P, NMAX * W], bf16, tag="rhs")
                    rhs3 = rhs[:, 0:nch * W].rearrange("p (j w) -> p j w", j=nch)
                    nc.scalar.activation(
                        out=rhs3[:, :, HID:W],
                        in_=alpha[:, 0:nch * heads].rearrange(
                            "p (j h) -> p j h", j=nch),
                        func=Act.Exp, scale=scale)
                    nc.vector.tensor_tensor(
                        out=rhs3[:, :, 0:HID].rearrange(
                            "p j (h d) -> p j h d", h=heads),
                        in0=vj[:, 0:FD].rearrange(
                            "p (j h d) -> p j h d", j=nch, h=heads),
                        in1=rhs3[:, :, HID:W].unsqueeze(3).to_broadcast(
                            [P, nch, heads, D]),
                        op=Alu.mult)

                    # segment sum via one-hot matmul, accumulated per tile
                    acc = psacc.tile([P, W], f32, space="PSUM", tag="acc")
                    for k in range(nch):
                        nc.tensor.matmul(
                            out=acc[:], lhsT=S_all[:, k * P:(k + 1) * P],
                            rhs=rhs[:, k * W:(k + 1) * W],
                            start=(k == 0), stop=(k == nch - 1))

                    # ---- node update
                    sb_t = npool.tile([P, heads], f32, tag="sb")
                    nc.vector.tensor_scalar_add(out=sb_t[:],
                                                in0=acc[:, HID:W], scalar1=1e-16)
                    rinv = npool.tile([P, heads], f32, tag="rinv")
                    nc.vector.reciprocal(out=rinv[:], in_=sb_t[:])
                    attn = npool.tile([P, HID], f32, tag="attn")
                    nc.vector.tensor_tensor(
                        out=attn[:].rearrange("p (h d) -> p h d", h=heads),
                        in0=acc[:, 0:HID].rearrange("p (h d) -> p h d", h=heads),
                        in1=rinv[:].unsqueeze(2).to_broadcast([P, heads, D]),
                        op=Alu.mult)
                    skt = pstage.tile([P, EGRP * HID], f32, space="PSUM",
                                      tag="stage")
                    sk = skt[:, 0:HID]
                    nc.tensor.matmul(out=sk, lhsT=xsk[:], rhs=wst[:],
                                     start=True, stop=True)
                    ht = npool.tile([P, HID], f32, tag="ht")
                    nc.vector.tensor_tensor(out=ht[:], in0=attn[:], in1=sk,
                                            op=Alu.add)
                    if relu:
                        ht2 = npool.tile([P, HID], bf16, tag="ht2")
                        nc.scalar.activation(out=ht2[:], in_=ht[:], func=Act.Lrelu,
                                             alpha=0.01)
                        tpt = pst.tile([P, EGRP * P], bf16, space="PSUM",
                                       tag="staget")
                        tp = tpt[:, 0:P]
                        nc.tensor.transpose(out=tp, in_=ht2[:], identity=idt[:])
                        hTt = npool.tile([P, P], bf16, tag="hTt")
                        nc.scalar.activation(out=hTt[:], in_=tp, func=Act.Copy)
                        nc.sync.dma_start(out=hT_loc[:, tl * P:(tl + 1) * P],
                                          in_=hTt[:])
                    else:
                        nc.sync.dma_start(out=out[tl * P:(tl + 1) * P, :],
                                          in_=ht[:])

                if allgather:
                    nc.gpsimd.collective_compute(
                        "AllGather", Alu.bypass,
                        replica_groups=[list(range(NCORES))],
                        ins=[hT_loc[:].opt()], outs=[hT_full[:].opt()])

    # layer-2 phase A reads the gathered hT (full) and local hT; kv batches
    # stay within one core's 49-tile block of hT_full
    def qkv_phase2():
        with tile.TileContext(nc) as tc:
            with tc.tile_pool(name="qb_c", bufs=1) as cst, \
                 tc.tile_pool(name="qb_s", bufs=3) as pool, \
                 tc.tile_pool(name="qb_p", bufs=2, space="PSUM") as psp:
                wt = cst.tile([HID, 3 * HID], bf16)
                nc.sync.dma_start(out=wt[:], in_=Wqkv2[:])
                for i0 in range(0, TPC, BA):
                    nb = min(BA, TPC - i0)
                    xt = pool.tile([P, BA * P], bf16, tag="xl")
                    nc.sync.dma_start(out=xt[:, 0:nb * P],
                                      in_=hT_loc[:, i0 * P:(i0 + nb) * P])
                    ps = psp.tile([P, BA * HID], f32, space="PSUM", tag="pq")
                    for j in range(nb):
                        nc.tensor.matmul(out=ps[:, j * HID:(j + 1) * HID],
                                         lhsT=xt[:, j * P:(j + 1) * P],
                                         rhs=wt[:, 0:HID], start=True, stop=True)
                    ev = pool.tile([P, BA * HID], bf16, tag="evq")
                    nc.scalar.activation(out=ev[:, 0:nb * HID],
                                         in_=ps[:, 0:nb * HID], func=Act.Copy)
                    nc.sync.dma_start(
                        out=q_loc[i0 * P:(i0 + nb) * P, :].rearrange(
                            "(j p) d -> p j d", j=nb),
                        in_=ev[:, 0:nb * HID].rearrange("p (j d) -> p j d", j=nb))
                for ci in range(NCORES):
                    for lt0 in range(0, TPC, BA):
                        nb = min(BA, TPC - lt0)
                        i0 = ci * TPC + lt0
                        xt = pool.tile([P, BA * P], bf16, tag="xf")
                        nc.sync.dma_start(
                            out=xt[:, 0:nb * P],
                            in_=hT_full[ci * HID:(ci + 1) * HID,
                                        lt0 * P:(lt0 + nb) * P])
                        ps = psp.tile([P, BA * 2 * HID], f32, space="PSUM",
                                      tag="pkv")
                        for j in range(nb):
                            nc.tensor.matmul(
                                out=ps[:, j * 2 * HID:(j + 1) * 2 * HID],
                                lhsT=xt[:, j * P:(j + 1) * P],
                                rhs=wt[:, HID:3 * HID], start=True, stop=True)
                        ev = pool.tile([P, BA * 2 * HID], bf16, tag="evkv")
                        nc.scalar.activation(out=ev[:, 0:nb * 2 * HID],
                                             in_=ps[:, 0:nb * 2 * HID],
                                             func=Act.Copy)
                        nc.sync.dma_start(
                            out=kv_tab[i0 * P:(i0 + nb) * P, :].rearrange(
                                "(j p) d -> p j d", j=nb),
                            in_=ev[:, 0:nb * 2 * HID].rearrange(
                                "p (j d) -> p j d", j=nb))

    qkv_phase(x_localT, x_fullT, Wqkv1)
    edge_phase(We1, Ws1, x_localT, heads=8, relu=True, allgather=True)
    qkv_phase2()
    edge_phase(We2, Ws2, hT_loc, heads=1, relu=False, allgather=False)

    _legalize_waits(nc)
    return nc


_CACHE = {}


def kernel(x, ei, ea, Wq1, bq1, Wk1, bk1, Wv1, bv1, We1, Ws1, bs1,
           Wq2, bq2, Wk2, bk2, Wv2, bv2, We2, Ws2, bs2):
    import ml_dtypes
    from concourse.bass_utils import run_bass_kernel_spmd
    bf = ml_dtypes.bfloat16

    for b in (bq1, bk1, bv1, bs1, bq2, bk2, bv2, bs2):
        assert not np.any(np.asarray(b)), "nonzero biases not supported"

    x = np.asarray(x, np.float32)
    x_pad = np.zeros((NP_, NODE_DIM), np.float32)
    x_pad[:N] = x
    x_fullT = np.ascontiguousarray(x_pad.T).astype(bf)
    cores, Cloc, off, NCH = _prep(np.asarray(ei), np.asarray(ea))

    key = (NCH, tuple(Cloc))
    if key not in _CACHE:
        _CACHE[key] = _build(Cloc, off, NCH)
    nc = _CACHE[key]

    def cat3(a, b, c):
        return np.ascontiguousarray(np.concatenate(
            [np.asarray(a, np.float32), np.asarray(b, np.float32),
             np.asarray(c, np.float32)], axis=1)).astype(bf)

    Wqkv1 = cat3(Wq1, Wk1, Wv1)
    Wqkv2 = cat3(Wq2, Wk2, Wv2)
    iota_in = np.tile(np.arange(P, dtype=np.float32)[None, :], (P, 1))
    ident_in = np.eye(P, dtype=np.float32).astype(bf)

    in_maps = []
    for c in range(NCORES):
        pc = cores[c]
        in_maps.append({
            "x_fullT": x_fullT,
            "x_localT": np.ascontiguousarray(
                x_fullT[:, c * NLC:(c + 1) * NLC]),
            "srcT": pc["srcT"], "dstrelT": pc["dstrelT"], "eaT": pc["eaT"],
            "iota_in": iota_in, "ident_in": ident_in,
            "Wqkv1": Wqkv1, "We1": np.asarray(We1, np.float32).astype(bf),
            "Ws1": np.asarray(Ws1, np.float32).astype(bf),
            "Wqkv2": Wqkv2, "We2": np.asarray(We2, np.float32).astype(bf),
            "Ws2": np.asarray(Ws2, np.float32).astype(bf),
        })
    res = run_bass_kernel_spmd(nc, in_maps, list(range(NCORES)))
    global LAST_RESULT
    LAST_RESULT = res
    out = np.concatenate([res.results[c]["out"] for c in range(NCORES)], axis=0)
    return np.ascontiguousarray(out[:N])


LAST_RESULT = None


# revision 17
# speedup vs baseline: 2.5411x; 1.0045x over previous
"""TransformerConv 2-layer GNN encoder on 8 Trainium2 NeuronCores (Bass/Tile).

v2 strategy (graph-partition parallel, bf16 tables, per-tile batching):
  - Nodes padded 50000 -> 50176 = 8 cores x 49 tiles x 128. Each core owns 49
    consecutive node tiles as TARGETS; edges assigned to the dst core, sorted
    by dst, packed into 128-edge chunks per tile (chunk counts equalized
    across cores so the SPMD program is identical).
  - Phase A (per layer): q for LOCAL tiles from x_localT (per-core input,
    SPMD-safe addressing); k|v for ALL tiles from x_fullT -> kv_tab
    [50176, 256] bf16. Host provides x transposed so no PE transposes needed.
  - Edge phase (per layer, per tile, batched over the tile's chunks):
      per chunk: ONE merged k|v indirect gather (512B rows, bf16);
      eps = ea@We on PE (4-chunk PSUM groups, single ACT evacuation);
      S one-hot [slot, c] built batched on DVE; ST = S^T via PE transpose
      (4-chunk PSUM groups); qg = ST^T@qtile on PE (q never gathered);
      batched DVE: kj=k+eps, vj=v+eps, prod=kj*qg, alpha=group-reduce,
      exp on ACT (straight into the rhs tile), vjw=vj*exp;
      segment softmax-sum via S^T@[vjw|exp] accumulated in PSUM per tile;
      fused divide + skip (PE) + lrelu; h stored transposed for layer 2.
  - One AllGather of hT (12.8MB bf16) between the layers.
Softmax: segment-max subtraction skipped (alphas are O(0.3); exact softmax
invariance) and the divide applied after summation - matches reference.
"""
import numpy as np

P = 128
N = 50000
NP_ = 50176
TILES = 392
NCORES = 8
TPC = TILES // NCORES          # 49 tiles per core
NLC = TPC * P                  # 6272 local nodes
NODE_DIM = 128
EDGE_DIM = 16
HID = 128
DSTREL_PAD = 200.0
EGRP = 4                       # chunks per PSUM staging group


# ----------------------------------------------------------------- host prep
def _prep(ei, ea):
    import ml_dtypes
    src = np.asarray(ei[0], dtype=np.int64)
    dst = np.asarray(ei[1], dtype=np.int64)
    ea = np.asarray(ea, dtype=np.float32)

    order = np.argsort(dst, kind="stable")
    src_s, dst_s, ea_s = src[order], dst[order], ea[order]

    tile_of = dst_s // P
    cnt = np.bincount(tile_of, minlength=TILES)
    C = (cnt + P - 1) // P
    Cloc = np.maximum(C.reshape(NCORES, TPC).max(axis=0), 1)   # [TPC]
    NCH = int(Cloc.sum())
    off = np.zeros(TPC, dtype=np.int64)
    off[1:] = np.cumsum(Cloc)[:-1]

    tile_starts = np.searchsorted(tile_of, np.arange(TILES))
    tile_ends = np.searchsorted(tile_of, np.arange(TILES), side="right")
    cores = []
    for c in range(NCORES):
        nslot = NCH * P
        src_sl = np.zeros(nslot, dtype=np.int32)
        drel_sl = np.full(nslot, DSTREL_PAD, dtype=np.float32)
        ea_sl = np.zeros((nslot, EDGE_DIM), dtype=np.float32)
        for tl in range(TPC):
            tg = c * TPC + tl
            a, b = tile_starts[tg], tile_ends[tg]
            if b == a:
                continue
            s0 = off[tl] * P
            src_sl[s0:s0 + b - a] = src_s[a:b]
            drel_sl[s0:s0 + b - a] = (dst_s[a:b] - tg * P).astype(np.float32)
            ea_sl[s0:s0 + b - a] = ea_s[a:b]
        cores.append(dict(
            srcT=np.ascontiguousarray(src_sl.reshape(NCH, P).T),
            dstrelT=np.ascontiguousarray(drel_sl.reshape(NCH, P).T),
            eaT=np.ascontiguousarray(ea_sl.T).astype(ml_dtypes.bfloat16),
        ))
    return cores, Cloc, off, NCH


# ------------------------------------------------------- walrus wait legalize
def _legalize_waits(nc):
    import concourse.mybir as mybir
    k = 0
    for bb in nc.main_func.blocks:
        il = bb.instructions
        new = []
        for ins in il:
            si = ins.sync_info
            if si is not None and len(si.on_wait) > 1:
                waits = list(si.on_wait)
                for w in waits[:-1]:
                    nop = mybir.InstNoOp(name=f"wn{k}-{ins.name}", ins=[], outs=[])
                    k += 1
                    nop.engine = ins.engine
                    nop.sync_info = mybir.SyncInfo(on_wait=[w], on_update=[])
                    new.append(nop)
                ins.sync_info = mybir.SyncInfo(on_wait=[waits[-1]],
                                               on_update=list(si.on_update))
            new.append(ins)
        il[:] = new


# ------------------------------------------------------------- device program
def _build(Cloc, off, NCH):
    import concourse.bass as bass
    import concourse.mybir as mybir
    import concourse.tile as tile
    f32 = mybir.dt.float32
    bf16 = mybir.dt.bfloat16
    i32 = mybir.dt.int32
    Alu = mybir.AluOpType
    Act = mybir.ActivationFunctionType

    nc = bass.Bass()
    dp = nc.declare_dram_parameter
    x_fullT = dp("x_fullT", [NODE_DIM, NP_], bf16, isOutput=False)
    x_localT = dp("x_localT", [NODE_DIM, NLC], bf16, isOutput=False)
    srcT = dp("srcT", [P, NCH], i32, isOutput=False)
    dstrelT = dp("dstrelT", [P, NCH], f32, isOutput=False)
    eaT = dp("eaT", [EDGE_DIM, NCH * P], bf16, isOutput=False)
    iota_in = dp("iota_in", [P, P], f32, isOutput=False)
    ident_in = dp("ident_in", [P, P], bf16, isOutput=False)
    Wqkv1 = dp("Wqkv1", [NODE_DIM, 3 * HID], bf16, isOutput=False)
    We1 = dp("We1", [EDGE_DIM, HID], bf16, isOutput=False)
    Ws1 = dp("Ws1", [NODE_DIM, HID], bf16, isOutput=False)
    Wqkv2 = dp("Wqkv2", [HID, 3 * HID], bf16, isOutput=False)
    We2 = dp("We2", [EDGE_DIM, HID], bf16, isOutput=False)
    Ws2 = dp("Ws2", [HID, HID], bf16, isOutput=False)
    out = dp("out", [NLC, HID], f32, isOutput=True)

    # kv rows stored as raw 512B; declared f32 so the indirect gather walks
    # 128 4-byte elements per row instead of 256 2-byte ones
    kv_tab = nc.dram_tensor("kv_tab", [NP_, HID], f32, kind="Internal")
    q_loc = nc.dram_tensor("q_loc", [NLC, HID], bf16, kind="Internal")
    hT_loc = nc.dram_tensor("hT_loc", [HID, NLC], bf16, kind="Internal")
    hT_full = nc.dram_tensor("hT_full", [NCORES * HID, NLC], bf16, kind="Internal")

    # ---------------- phase A: q for local tiles, k|v table for all tiles
    # 4-tile batches: one load DMA, 4 matmuls, one evacuation, one store DMA.
    BA = 4

    def _qkv_loops(tc, cst, pool, psp, wt, qsrcT, kvsrc_slice):
        for i0 in range(0, TPC, BA):
            nb = min(BA, TPC - i0)
            xt = pool.tile([P, BA * P], bf16, tag="xl")
            nc.sync.dma_start(out=xt[:, 0:nb * P],
                              in_=qsrcT(i0, nb))
            ps = psp.tile([P, BA * HID], f32, space="PSUM", tag="pq")
            for j in range(nb):
                nc.tensor.matmul(out=ps[:, j * HID:(j + 1) * HID],
                                 lhsT=xt[:, j * P:(j + 1) * P],
                                 rhs=wt[:, 0:HID], start=True, stop=True)
            ev = pool.tile([P, BA * HID], bf16, tag="evq")
            nc.scalar.activation(out=ev[:, 0:nb * HID], in_=ps[:, 0:nb * HID],
                                 func=Act.Copy)
            nc.sync.dma_start(
                out=q_loc[i0 * P:(i0 + nb) * P, :].rearrange(
                    "(j p) d -> p j d", j=nb),
                in_=ev[:, 0:nb * HID].rearrange("p (j d) -> p j d", j=nb))
        for i0 in range(0, TILES, BA):
            nb = min(BA, TILES - i0)
            xt = pool.tile([P, BA * P], bf16, tag="xf")
            nc.sync.dma_start(out=xt[:, 0:nb * P], in_=kvsrc_slice(i0, nb))
            ps = psp.tile([P, BA * 2 * HID], f32, space="PSUM", tag="pkv")
            for j in range(nb):
                nc.tensor.matmul(out=ps[:, j * 2 * HID:(j + 1) * 2 * HID],
                                 lhsT=xt[:, j * P:(j + 1) * P],
                                 rhs=wt[:, HID:3 * HID], start=True, stop=True)
            ev = pool.tile([P, BA * 2 * HID], bf16, tag="evkv")
            nc.scalar.activation(out=ev[:, 0:nb * 2 * HID],
                                 in_=ps[:, 0:nb * 2 * HID], func=Act.Copy)
            nc.sync.dma_start(
                out=kv_tab[i0 * P:(i0 + nb) * P, :].rearrange(
                    "(j p) d -> p j d", j=nb),
                in_=ev[:].bitcast(f32)[:, 0:nb * HID].rearrange(
                    "p (j d) -> p j d", j=nb))

    def qkv_phase(locT, fullT, wqkv):
        with tile.TileContext(nc) as tc:
            with tc.tile_pool(name="qa_c", bufs=1) as cst, \
                 tc.tile_pool(name="qa_s", bufs=3) as pool, \
                 tc.tile_pool(name="qa_p", bufs=2, space="PSUM") as psp:
                wt = cst.tile([NODE_DIM, 3 * HID], bf16)
                nc.sync.dma_start(out=wt[:], in_=wqkv[:])
                _qkv_loops(tc, cst, pool, psp, wt,
                           lambda i0, nb: locT[:, i0 * P:(i0 + nb) * P],
                           lambda i0, nb: fullT[:, i0 * P:(i0 + nb) * P])

    # ---------------- edge phase
    def edge_phase(we, ws, xlocT, heads, relu, allgather):
        D = HID // heads
        scale = 1.0 / float(np.sqrt(D))
        W = HID + heads
        NMAX = int(Cloc.max())
        with tile.TileContext(nc) as tc:
            with tc.tile_pool(name="eg_c", bufs=1) as cst, \
                 tc.tile_pool(name="eg_sl", bufs=3) as slp, \
                 tc.tile_pool(name="eg_g", bufs=4) as gp, \
                 tc.tile_pool(name="eg_w", bufs=2) as wp, \
                 tc.tile_pool(name="eg_n", bufs=2) as npool, \
                 tc.tile_pool(name="eg_ps", bufs=3, space="PSUM") as pstage, \
                 tc.tile_pool(name="eg_pt", bufs=2, space="PSUM") as pst, \
                 tc.tile_pool(name="eg_pa", bufs=2, space="PSUM") as psacc:
                iota_f = cst.tile([P, P], f32)
                nc.sync.dma_start(out=iota_f[:], in_=iota_in[:])
                idt = cst.tile([P, P], bf16)
                nc.sync.dma_start(out=idt[:], in_=ident_in[:])
                wet = cst.tile([EDGE_DIM, HID], bf16)
                nc.sync.dma_start(out=wet[:], in_=we[:])
                wst = cst.tile([HID, HID], bf16)
                nc.sync.dma_start(out=wst[:], in_=ws[:])

                for tl in range(TPC):
                    nch = int(Cloc[tl])
                    c0 = int(off[tl])
                    FD = nch * P
                    ngr = (nch + EGRP - 1) // EGRP

                    ssl = slp.tile([P, NMAX], i32, tag="ssl")
                    nc.sync.dma_start(out=ssl[:, 0:nch], in_=srcT[:, c0:c0 + nch])
                    dsl = slp.tile([P, NMAX], f32, tag="dsl")
                    nc.sync.dma_start(out=dsl[:, 0:nch], in_=dstrelT[:, c0:c0 + nch])
                    ea_all = slp.tile([EDGE_DIM, NMAX * P], bf16, tag="ea")
                    nc.sync.dma_start(out=ea_all[:, 0:FD],
                                      in_=eaT[:, c0 * P:(c0 + nch) * P])
                    qtile = slp.tile([P, HID], bf16, tag="qt")
                    nc.sync.dma_start(out=qtile[:],
                                      in_=q_loc[tl * P:(tl + 1) * P, :])
                    xsk = slp.tile([P, P], bf16, tag="xsk")
                    nc.sync.dma_start(out=xsk[:], in_=xlocT[:, tl * P:(tl + 1) * P])

                    # merged k|v gathers, one per chunk (f32-typed raw rows)
                    kvg_r = gp.tile([P, NMAX * HID], f32, tag="kvg")
                    for k in range(nch):
                        nc.gpsimd.indirect_dma_start(
                            out=kvg_r[:, k * HID:(k + 1) * HID],
                            out_offset=None, in_=kv_tab[:],
                            in_offset=bass.IndirectOffsetOnAxis(
                                ap=ssl[:, k:k + 1], axis=0))
                    kvg = kvg_r[:].bitcast(bf16)

                    # S one-hot [slot, (j, c)] batched
                    S_all = wp.tile([P, NMAX * P], bf16, tag="S")
                    nc.vector.tensor_tensor(
                        out=S_all[:, 0:FD].rearrange("p (j c) -> p j c", j=nch),
                        in0=dsl[:, 0:nch].unsqueeze(2).to_broadcast([P, nch, P]),
                        in1=iota_f[:].unsqueeze(1).to_broadcast([P, nch, P]),
                        op=Alu.is_equal)

                    # eps / ST / qg staged through PSUM in EGRP-chunk groups
                    eps_sb = wp.tile([P, NMAX * HID], bf16, tag="eps")
                    st_sb = wp.tile([P, NMAX * P], bf16, tag="st")
                    qg_sb = wp.tile([P, NMAX * HID], bf16, tag="qg")
                    for g in range(ngr):
                        k0, k1 = g * EGRP, min(nch, (g + 1) * EGRP)
                        nk = k1 - k0
                        pe = pstage.tile([P, EGRP * HID], f32, space="PSUM",
                                         tag="stage")
                        for k in range(k0, k1):
                            j = k - k0
                            nc.tensor.matmul(
                                out=pe[:, j * HID:(j + 1) * HID],
                                lhsT=ea_all[:, k * P:(k + 1) * P],
                                rhs=wet[:], start=True, stop=True)
                        nc.scalar.activation(out=eps_sb[:, k0 * HID:k1 * HID],
                                             in_=pe[:, 0:nk * HID], func=Act.Copy)
                        pt = pst.tile([P, EGRP * P], bf16, space="PSUM",
                                      tag="staget")
                        for k in range(k0, k1):
                            j = k - k0
                            nc.tensor.transpose(
                                out=pt[:, j * P:(j + 1) * P],
                                in_=S_all[:, k * P:(k + 1) * P], identity=idt[:])
                        nc.scalar.activation(out=st_sb[:, k0 * P:k1 * P],
                                             in_=pt[:, 0:nk * P], func=Act.Copy)
                        pq = pstage.tile([P, EGRP * HID], f32, space="PSUM",
                                         tag="stage")
                        for k in range(k0, k1):
                            j = k - k0
                            nc.tensor.matmul(
                                out=pq[:, j * HID:(j + 1) * HID],
                                lhsT=st_sb[:, k * P:(k + 1) * P],
                                rhs=qtile[:], start=True, stop=True)
                        nc.scalar.activation(out=qg_sb[:, k0 * HID:k1 * HID],
                                             in_=pq[:, 0:nk * HID], func=Act.Copy)

                    # batched DVE: kj, vj, prod, alpha
                    kj = wp.tile([P, NMAX * HID], bf16, tag="kj")
                    nc.vector.tensor_tensor(
                        out=kj[:, 0:FD].rearrange("p (j d) -> p j d", j=nch),
                        in0=kvg[:, 0:nch * 2 * HID].rearrange(
                            "p (j d) -> p j d", j=nch)[:, :, 0:HID],
                        in1=eps_sb[:, 0:FD].rearrange("p (j d) -> p j d", j=nch),
                        op=Alu.add)
                    vj = wp.tile([P, NMAX * HID], bf16, tag="vj")
                    nc.vector.tensor_tensor(
                        out=vj[:, 0:FD].rearrange("p (j d) -> p j d", j=nch),
                        in0=kvg[:, 0:nch * 2 * HID].rearrange(
                            "p (j d) -> p j d", j=nch)[:, :, HID:2 * HID],
                        in1=eps_sb[:, 0:FD].rearrange("p (j d) -> p j d", j=nch),
                        op=Alu.add)
                    prod = wp.tile([P, NMAX * HID], bf16, tag="prod")
                    nc.vector.tensor_tensor(out=prod[:, 0:FD], in0=kj[:, 0:FD],
                                            in1=qg_sb[:, 0:FD], op=Alu.mult)
                    alpha = wp.tile([P, NMAX * 8], f32, tag="alpha")
                    nc.vector.tensor_reduce(
                        out=alpha[:, 0:nch * heads],
                        in_=prod[:, 0:FD].rearrange("p (g d) -> p g d", d=D),
                        axis=mybir.AxisListType.X, op=Alu.add)

                    # rhs = [vj*exp | exp]
                    rhs = wp.tile([P, NMAX * W], bf16, tag="rhs")
                    rhs3 = rhs[:, 0:nch * W].rearrange("p (j w) -> p j w", j=nch)
                    nc.scalar.activation(
                        out=rhs3[:, :, HID:W],
                        in_=alpha[:, 0:nch * heads].rearrange(
                            "p (j h) -> p j h", j=nch),
                        func=Act.Exp, scale=scale)
                    nc.vector.tensor_tensor(
                        out=rhs3[:, :, 0:HID].rearrange(
                            "p j (h d) -> p j h d", h=heads),
                        in0=vj[:, 0:FD].rearrange(
                            "p (j h d) -> p j h d", j=nch, h=heads),
                        in1=rhs3[:, :, HID:W].unsqueeze(3).to_broadcast(
                            [P, nch, heads, D]),
                        op=Alu.mult)

                    # segment sum via one-hot matmul, accumulated per tile
                    acc = psacc.tile([P, W], f32, space="PSUM", tag="acc")
                    for k in range(nch):
                        nc.tensor.matmul(
                            out=acc[:], lhsT=S_all[:, k * P:(k + 1) * P],
                            rhs=rhs[:, k * W:(k + 1) * W],
                            start=(k == 0), stop=(k == nch - 1))

                    # ---- node update
                    sb_t = npool.tile([P, heads], f32, tag="sb")
                    nc.vector.tensor_scalar_add(out=sb_t[:],
                                                in0=acc[:, HID:W], scalar1=1e-16)
                    rinv = npool.tile([P, heads], f32, tag="rinv")
                    nc.vector.reciprocal(out=rinv[:], in_=sb_t[:])
                    attn = npool.tile([P, HID], f32, tag="attn")
                    nc.vector.tensor_tensor(
                        out=attn[:].rearrange("p (h d) -> p h d", h=heads),
                        in0=acc[:, 0:HID].rearrange("p (h d) -> p h d", h=heads),
                        in1=rinv[:].unsqueeze(2).to_broadcast([P, heads, D]),
                        op=Alu.mult)
                    skt = pstage.tile([P, EGRP * HID], f32, space="PSUM",
                                      tag="stage")
                    sk = skt[:, 0:HID]
                    nc.tensor.matmul(out=sk, lhsT=xsk[:], rhs=wst[:],
                                     start=True, stop=True)
                    ht = npool.tile([P, HID], f32, tag="ht")
                    nc.vector.tensor_tensor(out=ht[:], in0=attn[:], in1=sk,
                                            op=Alu.add)
                    if relu:
                        ht2 = npool.tile([P, HID], bf16, tag="ht2")
                        nc.scalar.activation(out=ht2[:], in_=ht[:], func=Act.Lrelu,
                                             alpha=0.01)
                        tpt = pst.tile([P, EGRP * P], bf16, space="PSUM",
                                       tag="staget")
                        tp = tpt[:, 0:P]
                        nc.tensor.transpose(out=tp, in_=ht2[:], identity=idt[:])
                        hTt = npool.tile([P, P], bf16, tag="hTt")
                        nc.scalar.activation(out=hTt[:], in_=tp, func=Act.Copy)
                        nc.sync.dma_start(out=hT_loc[:, tl * P:(tl + 1) * P],
                                          in_=hTt[:])
                    else:
                        nc.sync.dma_start(out=out[tl * P:(tl + 1) * P, :],
                                          in_=ht[:])

                if allgather:
                    nc.gpsimd.collective_compute(
                        "AllGather", Alu.bypass,
                        replica_groups=[list(range(NCORES))],
                        ins=[hT_loc[:].opt()], outs=[hT_full[:].opt()])

    # layer-2 phase A reads the gathered hT (full) and local hT; kv batches
    # stay within one core's 49-tile block of hT_full
    def qkv_phase2():
        with tile.TileContext(nc) as tc:
            with tc.tile_pool(name="qb_c", bufs=1) as cst, \
                 tc.tile_pool(name="qb_s", bufs=3) as pool, \
                 tc.tile_pool(name="qb_p", bufs=2, space="PSUM") as psp:
                wt = cst.tile([HID, 3 * HID], bf16)
                nc.sync.dma_start(out=wt[:], in_=Wqkv2[:])
                for i0 in range(0, TPC, BA):
                    nb = min(BA, TPC - i0)
                    xt = pool.tile([P, BA * P], bf16, tag="xl")
                    nc.sync.dma_start(out=xt[:, 0:nb * P],
                                      in_=hT_loc[:, i0 * P:(i0 + nb) * P])
                    ps = psp.tile([P, BA * HID], f32, space="PSUM", tag="pq")
                    for j in range(nb):
                        nc.tensor.matmul(out=ps[:, j * HID:(j + 1) * HID],
                                         lhsT=xt[:, j * P:(j + 1) * P],
                                         rhs=wt[:, 0:HID], start=True, stop=True)
                    ev = pool.tile([P, BA * HID], bf16, tag="evq")
                    nc.scalar.activation(out=ev[:, 0:nb * HID],
                                         in_=ps[:, 0:nb * HID], func=Act.Copy)
                    nc.sync.dma_start(
                        out=q_loc[i0 * P:(i0 + nb) * P, :].rearrange(
                            "(j p) d -> p j d", j=nb),
                        in_=ev[:, 0:nb * HID].rearrange("p (j d) -> p j d", j=nb))
                for ci in range(NCORES):
                    for lt0 in range(0, TPC, BA):
                        nb = min(BA, TPC - lt0)
                        i0 = ci * TPC + lt0
                        xt = pool.tile([P, BA * P], bf16, tag="xf")
                        nc.sync.dma_start(
                            out=xt[:, 0:nb * P],
                            in_=hT_full[ci * HID:(ci + 1) * HID,
                                        lt0 * P:(lt0 + nb) * P])
                        ps = psp.tile([P, BA * 2 * HID], f32, space="PSUM",
                                      tag="pkv")
                        for j in range(nb):
                            nc.tensor.matmul(
                                out=ps[:, j * 2 * HID:(j + 1) * 2 * HID],
                                lhsT=xt[:, j * P:(j + 1) * P],
                                rhs=wt[:, HID:3 * HID], start=True, stop=True)
                        ev = pool.tile([P, BA * 2 * HID], bf16, tag="evkv")
                        nc.scalar.activation(out=ev[:, 0:nb * 2 * HID],
                                             in_=ps[:, 0:nb * 2 * HID],
                                             func=Act.Copy)
                        nc.sync.dma_start(
                            out=kv_tab[i0 * P:(i0 + nb) * P, :].rearrange(
                                "(j p) d -> p j d", j=nb),
                            in_=ev[:].bitcast(f32)[:, 0:nb * HID].rearrange(
                                "p (j d) -> p j d", j=nb))

    qkv_phase(x_localT, x_fullT, Wqkv1)
    edge_phase(We1, Ws1, x_localT, heads=8, relu=True, allgather=True)
    qkv_phase2()
    edge_phase(We2, Ws2, hT_loc, heads=1, relu=False, allgather=False)

    _legalize_waits(nc)
    return nc


_CACHE = {}


def kernel(x, ei, ea, Wq1, bq1, Wk1, bk1, Wv1, bv1, We1, Ws1, bs1,
           Wq2, bq2, Wk2, bk2, Wv2, bv2, We2, Ws2, bs2):
    import ml_dtypes
    from concourse.bass_utils import run_bass_kernel_spmd
    bf = ml_dtypes.bfloat16

    for b in (bq1, bk1, bv1, bs1, bq2, bk2, bv2, bs2):
        assert not np.any(np.asarray(b)), "nonzero biases not supported"

    x = np.asarray(x, np.float32)
    x_pad = np.zeros((NP_, NODE_DIM), np.float32)
    x_pad[:N] = x
    x_fullT = np.ascontiguousarray(x_pad.T).astype(bf)
    cores, Cloc, off, NCH = _prep(np.asarray(ei), np.asarray(ea))

    key = (NCH, tuple(Cloc))
    if key not in _CACHE:
        _CACHE[key] = _build(Cloc, off, NCH)
    nc = _CACHE[key]

    def cat3(a, b, c):
        return np.ascontiguousarray(np.concatenate(
            [np.asarray(a, np.float32), np.asarray(b, np.float32),
             np.asarray(c, np.float32)], axis=1)).astype(bf)

    Wqkv1 = cat3(Wq1, Wk1, Wv1)
    Wqkv2 = cat3(Wq2, Wk2, Wv2)
    iota_in = np.tile(np.arange(P, dtype=np.float32)[None, :], (P, 1))
    ident_in = np.eye(P, dtype=np.float32).astype(bf)

    in_maps = []
    for c in range(NCORES):
        pc = cores[c]
        in_maps.append({
            "x_fullT": x_fullT,
            "x_localT": np.ascontiguousarray(
                x_fullT[:, c * NLC:(c + 1) * NLC]),
            "srcT": pc["srcT"], "dstrelT": pc["dstrelT"], "eaT": pc["eaT"],
            "iota_in": iota_in, "ident_in": ident_in,
            "Wqkv1": Wqkv1, "We1": np.asarray(We1, np.float32).astype(bf),
            "Ws1": np.asarray(Ws1, np.float32).astype(bf),
            "Wqkv2": Wqkv2, "We2": np.asarray(We2, np.float32).astype(bf),
            "Ws2": np.asarray(Ws2, np.float32).astype(bf),
        })
    res = run_bass_kernel_spmd(nc, in_maps, list(range(NCORES)))
    global LAST_RESULT
    LAST_RESULT = res
    out = np.concatenate([res.results[c]["out"] for c in range(NCORES)], axis=0)
    return np.ascontiguousarray(out[:N])


LAST_RESULT = None
